# revision 15
# baseline (speedup 1.0000x reference)
"""GRUAggregation1d Trainium2 kernel.

Computes, for xs [B=16, 512, L=8192], z_prev [B, 128, L] (all fp32):
    q  = sigmoid(Wq@xs + Uq@z + bq)        (per position l, batch b)
    r  = sigmoid(Wr@xs + Ur@z + br)
    zt = tanh(Wz@xs + Uz@(r*z) + bz)
    out = q*z + (1-q)*zt

Sharding: data-parallel over batch. 8 cores x 2 batches each; weights
replicated.

Design (v11 ~116.7us -> this v13):
- q/zt matmuls bf16 (fp8 measured out of tolerance on those paths); the
  r gate runs fp8 DoubleRow (K=256 per pass) with x64 pre-scaled weights,
  compensated via the sigmoid's scale; Ur is pre-scaled x64 in bf16 so
  its products land on the same PSUM scale. PSUM fp32, N=512 per matmul.
- Work unit: 1024-position supertile == DMA chunk. 3-chunk input
  prefetch; xs/z on the sync HW queue, out-stores on gpsimd. The fp8
  copy of xs is derived ON-CHIP, split 3/4 on DVE (tensor-scalar cast,
  k-chunks 0-2, 1.7us) and 1/4 on ACT (copy, k-chunk 3, 1.2us): v11 did
  the whole cast on DVE (2.3us), which put DVE at 62% busy and caused a
  1.9us PE stall + HAM re-throttle when cast+rz+combine fell behind.
- Chunk 0 is latency-tuned: the weight pack is reordered [Wr|Ur|Wq|Uq|
  Wz|Uz] and DMA'd in two pieces (Wr+Ur 160KB first) so LDWEIGHTS fires
  at ~9.4us; chunk 0's xs arrives as k-quarter DMAs, k0-k2 on sync
  interleaved with z, k3 on the scalar HW queue (its only bulk use —
  the ring is ~123 GB/s, fine for a one-off 256KB). Chunk 0's r gate
  runs k-major on the bf16 Wr (x64 pre-scaled like the fp8 twin), with
  Ur accumulated mid-stream (stop on the k3 matmuls) as soon as z lands.
- Per supertile the r gate is computed FIRST so r*z (DVE, bf16) is
  ready ~1.3us before the Uz matmuls at the stream tail.
- Consecutive matmuls always target different PSUM banks; the two
  half-tile matmuls per weight are adjacent (stationary-weight reuse).
- ACT ops are 1024 wide with fused bias (PSUM fp32 in, bf16 out); the
  combine runs in bf16 on DVE (2x rate); z_prev and out are bf16 in HBM
  (host casts), halving that traffic. ACT stream per supertile: sig_r,
  sig_q, cast_k3(n+2), tanh — the cast sits in ACT's idle window, off
  the sig_r -> rz critical chain.
- The last chunk's two half-stores go to the scalar+sync HW queues in
  parallel; the gpsimd software queue (~120-150 GB/s + 2.7us drain) was
  the tail straggler.

Measured dead ends (do not retry without new evidence; noise is ~+-1us):
- fp8 (even with xs hi+lo residual splitting) on the q or tanh paths:
  2.1e-2..3.3e-2 max rel err vs the 2e-2 gate. Only the r gate tolerates
  fp8 (sigmoid'-damped, then averaged through Uz and tanh).
- DoubleRow streams 1024 column-slots per K=256 matmul on this silicon
  (~430ns); the win comes from two DR matmuls to different PSUM banks
  running concurrently (~600ns/pair = 1.44x). Permuting the rhs AP to
  [p, n, 2] fails BIR verification (pair must be dim1).
- bf16 matmuls do NOT pair-overlap (trace: strict 216ns N=512 spacing
  even across PSUM banks); the stream floor is ~5.95us/supertile.
- HOST-SHIPPED x8 (v12): +10.6us. The scalar HW ring is only ~123 GB/s
  while busy, and total input (33.6MB/core) hits the ~358 GB/s HBM/NC
  ceiling: the x8 stream starved, the PE stalled ~11us total and HAM
  re-throttled mid-kernel. x8 must be derived on-chip.
- 2048-position DMA granularity (2 supertiles/load): +9us. Emitting
  x8/xs triggers in swapped order: +5us.
- GpSimd for the bulk fp8 cast: ~70us per [128,4096] op (30x too slow).
"""

from contextlib import ExitStack

import ml_dtypes
import numpy as np

import concourse.bass as bass
import concourse.mybir as mybir
import concourse.tile as tile
from concourse import bacc
from concourse.bass_utils import run_bass_kernel_spmd

B, IN_DIM, WIDTH, L = 16, 512, 128, 8192
N_CORES = 8
B_PER = B // N_CORES          # batches per core
KC = IN_DIM // 128            # K chunks for the W matmuls
NT = 512                      # positions per matmul (one PSUM bank)
ST = 1024                     # supertile / DMA chunk positions
F32 = mybir.dt.float32
BF16 = mybir.dt.bfloat16
FP8 = mybir.dt.float8e4
WRS = 64.0                    # r-gate fp8 weight pre-scale
WP_FIRST = 640                # Wr + Ur columns, shipped first

_module_cache = {}


def _build():
    key = ("v15", ST)
    if key in _module_cache:
        return _module_cache[key]

    nc = bacc.Bacc("TRN2", target_bir_lowering=False, debug=False,
                   num_devices=N_CORES)

    xs_d = nc.dram_tensor("xs", [B_PER, IN_DIM, L], BF16, kind="ExternalInput").ap()
    zp_d = nc.dram_tensor("zp", [B_PER, WIDTH, L], BF16, kind="ExternalInput").ap()
    wp_d = nc.dram_tensor("wp", [128, 1920], BF16, kind="ExternalInput").ap()
    w8_d = nc.dram_tensor("w8", [128, 2, 2, 128], FP8, kind="ExternalInput").ap()
    bp_d = nc.dram_tensor("bp", [128, 3], F32, kind="ExternalInput").ap()
    out_d = nc.dram_tensor("out", [B_PER, WIDTH, L], BF16,
                           kind="ExternalOutput").ap()

    # [b, (k p), l] -> [b, p, k, l]: a chunk slice is a [128, KC, ST] DMA
    # with contiguous rows
    xs_r = xs_d.rearrange("b (k p) l -> b p k l", p=128)

    with tile.TileContext(nc) as tc, ExitStack() as ctx:
        wpool = ctx.enter_context(tc.tile_pool(name="weights", bufs=1))
        io = ctx.enter_context(tc.tile_pool(name="io", bufs=5))
        obuf = ctx.enter_context(tc.tile_pool(name="obuf", bufs=2))
        acts = ctx.enter_context(tc.tile_pool(name="acts", bufs=2))
        ps_q = ctx.enter_context(tc.tile_pool(name="ps_q", bufs=2,
                                              space="PSUM"))
        ps_rz = ctx.enter_context(tc.tile_pool(name="ps_rz", bufs=1,
                                               space="PSUM"))

        w_sb = wpool.tile([128, 1920], BF16, tag="wp")
        w8_sb = wpool.tile([128, 2, 2, 128], FP8, tag="w8")
        b_sb = wpool.tile([128, 3], F32, tag="bp")
        # Wr+Ur first on sync (LDWEIGHTS at ~9.4us); w8 on gpsimd; bias
        # on scalar.  wp_rest follows chunk 0's sync-side inputs.
        nc.sync.dma_start(w_sb[:, 0:WP_FIRST], wp_d[:, 0:WP_FIRST])
        nc.gpsimd.dma_start(w8_sb[:], w8_d[:])
        nc.scalar.dma_start(b_sb[:], bp_d[:])

        # PE warmup: ~6 dummy matmuls on a zeroed scratch tile fill the
        # HAM activity window during the DMA fill, so the real stream
        # starts at 2.4 GHz instead of paying ~2.5us of 1.2 GHz matmuls
        # (HAM un-throttles only after ~3.4us of sustained PE busy).
        warm = wpool.tile([128, NT], BF16, tag="warm")
        nc.vector.memzero(warm[:])
        warm_ps = ps_q.tile([128, ST], F32, tag="q_ps")
        for i in range(11):
            h = i % 2
            nc.tensor.matmul(warm_ps[:, h * NT:(h + 1) * NT],
                             warm[:, 0:128], warm[:], start=True, stop=True)

        # wp layout: [Wr 512 | Ur 128 | Wq 512 | Uq 128 | Wz 512 | Uz 128]
        def wr_slice(k):
            return w_sb[:, k * 128:(k + 1) * 128]

        def wq_slice(k):
            return w_sb[:, 640 + k * 128: 640 + (k + 1) * 128]

        def wz_slice(k):
            return w_sb[:, 1280 + k * 128: 1280 + (k + 1) * 128]

        def uslice(u):  # 0=r, 1=q, 2=z
            off = (512, 1152, 1792)[u]
            return w_sb[:, off: off + 128]

        n_chunks = B_PER * (L // ST)
        tiles = {}

        def load_chunk(m):
            """Input DMAs for chunk m. The fp8 tile is allocated here but
            filled later by DVE (k0-2) + ACT (k3) casts, ~2 chunks ahead
            of use."""
            mb, ml = divmod(m, L // ST)
            ml *= ST
            xs_t = io.tile([128, KC, ST], BF16, tag="xs_t")
            z_t = io.tile([128, ST], BF16, tag="z_t")
            if m == 0:
                # chunk 0 is DMA-sem-paced (~2us completion latency per
                # piece, early queue ~130 GB/s): k0, k1 + z on sync with
                # wp_rest slotted between, k2+k3 on the scalar HW queue.
                nc.sync.dma_start(xs_t[:, 0], xs_r[mb][:, 0, ml:ml + ST])
                nc.sync.dma_start(w_sb[:, WP_FIRST:1920],
                                  wp_d[:, WP_FIRST:1920])
                nc.sync.dma_start(xs_t[:, 1], xs_r[mb][:, 1, ml:ml + ST])
                nc.sync.dma_start(z_t[:], zp_d[mb][:, ml:ml + ST])
                for k in (2, 3):
                    nc.scalar.dma_start(xs_t[:, k],
                                        xs_r[mb][:, k, ml:ml + ST])
            else:
                nc.sync.dma_start(xs_t[:], xs_r[mb][:, :, ml:ml + ST])
                nc.sync.dma_start(z_t[:], zp_d[mb][:, ml:ml + ST])
            x8_t = None
            if m > 1:  # chunks 0-1's r gate runs on the bf16 weights
                x8_t = io.tile([128, KC, ST], FP8, tag="x8_t")
            tiles[m] = (xs_t, x8_t, z_t)

        LOOKAHEAD = 2
        for m in range(min(LOOKAHEAD + 1, n_chunks)):
            load_chunk(m)

        for n in range(n_chunks):
            if n + LOOKAHEAD + 1 < n_chunks:
                load_chunk(n + LOOKAHEAD + 1)
            b_i, l0 = divmod(n, L // ST)
            l0 *= ST
            xs_t, x8_t, z_t = tiles.pop(n)
            ob = obuf.tile([128, ST], BF16, tag="ob")

            q_ps = ps_q.tile([128, ST], F32, tag="q_ps")
            r_ps = ps_rz.tile([128, ST], F32, tag="r_ps")
            zt_ps = ps_rz.tile([128, ST], F32, tag="zt_ps")

            def r_w_bf16(k, start):
                for h in range(2):
                    nc.tensor.matmul(r_ps[:, h * NT:(h + 1) * NT],
                                     wr_slice(k),
                                     xs_t[:, k, h * NT:(h + 1) * NT],
                                     start=start, stop=False)

            def q_w(k):
                for h in range(2):
                    nc.tensor.matmul(q_ps[:, h * NT:(h + 1) * NT],
                                     wq_slice(k),
                                     xs_t[:, k, h * NT:(h + 1) * NT],
                                     start=(k == 0), stop=False)

            def u_mm(ps, u, h, rhs, stop=True):
                nc.tensor.matmul(ps[:, h * NT:(h + 1) * NT], uslice(u),
                                 rhs[:, h * NT:(h + 1) * NT],
                                 start=False, stop=stop)

            if n == 0:
                # ordered by DMA-completion estimates: sync carries
                # [wp_f, k0, wp_rest, k1, z], scalar [bp, k2, k3]
                r_w_bf16(0, True)
                r_w_bf16(2, False)
                q_w(0)
                q_w(2)
                q_w(1)
                q_w(3)
                r_w_bf16(3, False)
                r_w_bf16(1, False)
                for h in range(2):
                    u_mm(r_ps, 0, h, z_t)
                for h in range(2):
                    u_mm(q_ps, 1, h, z_t)
            elif n == 1:
                # bf16 r as well: skipping the x8 cast for chunk 1 keeps
                # DVE's chunk-0 window free and avoids a cast-gated stall
                # at the 0->1 boundary.
                for k in range(KC):
                    r_w_bf16(k, k == 0)
                for h in range(2):
                    u_mm(r_ps, 0, h, z_t)
                for k in range(KC):
                    q_w(k)
                for h in range(2):
                    u_mm(q_ps, 1, h, z_t)
            else:
                # The PE dual-issues a DoubleRow matmul alongside an
                # in-flight bf16 matmul when they target different PSUM
                # banks (~215ns marginal cost vs 600ns/pair standalone).
                # Weave the 4 r DR matmuls into the q stream, with each
                # Ur emitted right after its bank's last DR so sig_r can
                # release r_ps early for the next supertile.
                dr = [(0, 0), (0, 1), (1, 0), (1, 1)]
                for k in range(KC):
                    k2, dh = dr[k]
                    nc.tensor.matmul(
                        r_ps[:, dh * NT:(dh + 1) * NT], w8_sb[:, k2],
                        x8_t[:, 2 * k2:2 * k2 + 2, dh * NT:(dh + 1) * NT],
                        start=(k2 == 0), stop=False,
                        perf_mode=mybir.MatmulPerfMode.DoubleRow)
                    q_w(k)
                    if k == 2:
                        u_mm(r_ps, 0, 0, z_t)   # bank A stop after DR10
                if True:
                    u_mm(r_ps, 0, 1, z_t)       # bank B stop after DR11
                    for h in range(2):
                        u_mm(q_ps, 1, h, z_t)
            # sig_r + rz run in 512-halves: each r_ps bank is released
            # to the next supertile's DR matmuls as soon as ITS half is
            # read, and rz streams to the Uz tail sooner.
            r_s = acts.tile([128, ST], BF16, tag="r_s")
            rz = acts.tile([128, ST], BF16, tag="rz")
            for h in range(2):
                nc.scalar.activation(r_s[:, h * NT:(h + 1) * NT],
                                     r_ps[:, h * NT:(h + 1) * NT],
                                     mybir.ActivationFunctionType.Sigmoid,
                                     bias=b_sb[:, 1:2], scale=1.0 / WRS)
            for h in range(2):
                nc.vector.tensor_mul(rz[:, h * NT:(h + 1) * NT],
                                     r_s[:, h * NT:(h + 1) * NT],
                                     z_t[:, h * NT:(h + 1) * NT])
            # bf16->fp8 cast of chunk n+2's xs, split DVE (k0-2) + ACT
            # (k3), each in its idle window, always >=2 chunks ahead of
            # the fp8 r matmuls that consume it.
            cm = n + 2
            cast = (cm < n_chunks
                    and tiles.get(cm, (None, None))[1] is not None)
            if cast:
                nc.vector.tensor_scalar_add(tiles[cm][1][:, 0:3],
                                            tiles[cm][0][:, 0:3], 0.0)
            q_s = acts.tile([128, ST], BF16, tag="q_s")
            nc.scalar.activation(q_s[:], q_ps[:],
                                 mybir.ActivationFunctionType.Sigmoid,
                                 bias=b_sb[:, 0:1])
            if cast:
                nc.scalar.copy(tiles[cm][1][:, 3], tiles[cm][0][:, 3])

            # ---- zt gate: W part, then Uz@(r*z) at the stream tail
            for k in range(KC):
                w = wz_slice(k)
                for h in range(2):
                    nc.tensor.matmul(
                        zt_ps[:, h * NT:(h + 1) * NT], w,
                        xs_t[:, k, h * NT:(h + 1) * NT],
                        start=(k == 0), stop=False)
            uz = uslice(2)
            for h in range(2):
                nc.tensor.matmul(zt_ps[:, h * NT:(h + 1) * NT], uz,
                                 rz[:, h * NT:(h + 1) * NT],
                                 start=False, stop=True)

            # ---- epilogue: out = zt + q*(z - zt), bf16 on DVE. The last
            # chunk runs it in 512-halves so the serial tail chain
            # (tanh -> sub -> mul -> add -> store) pipelines, with the
            # half-stores split over the two fast HW queues.
            if n < n_chunks - 1:
                pieces = ((0, ST),)
            else:
                pieces = ((0, NT), (NT, NT))
            for pi, (h0, hw_) in enumerate(pieces):
                zt_s = acts.tile([128, hw_], BF16, tag=f"zt_s{h0}")
                nc.scalar.activation(zt_s[:], zt_ps[:, h0:h0 + hw_],
                                     mybir.ActivationFunctionType.Tanh,
                                     bias=b_sb[:, 2:3])
                diff = acts.tile([128, hw_], BF16, tag=f"diff{h0}")
                nc.vector.tensor_sub(diff[:], z_t[:, h0:h0 + hw_], zt_s[:])
                prod = acts.tile([128, hw_], BF16, tag=f"prod{h0}")
                nc.vector.tensor_mul(prod[:], q_s[:, h0:h0 + hw_], diff[:])
                nc.vector.tensor_add(ob[:, h0:h0 + hw_], zt_s[:], prod[:])
                if n < n_chunks - 1:
                    eng = nc.gpsimd
                else:
                    eng = nc.scalar if pi % 2 == 0 else nc.sync
                eng.dma_start(out_d[b_i][:, l0 + h0:l0 + h0 + hw_],
                              ob[:, h0:h0 + hw_])

    nc.compile()
    _module_cache[key] = nc
    return nc


def _pack_weights(inputs):
    # wp [128, 1920] bf16, layout [Wr|Ur|Wq|Uq|Wz|Uz]; per partition p:
    #   W at [p, off+k*128+o] = Wg_w[o, k*128+p]; U at [p, off+o] =
    #   Ug_w[o, p].  Wr/Ur pre-scaled by WRS (fp8-path compensation).
    wp = np.empty((128, 1920), np.float32)
    bp = np.empty((128, 3), np.float32)
    for i, (wn, un, wbn, ubn, ws) in enumerate((
        ("Wr_w", "Ur_w", "Wr_b", "Ur_b", WRS),
        ("Wq_w", "Uq_w", "Wq_b", "Uq_b", 1.0),
        ("Wz_w", "Uz_w", "Wz_b", "Uz_b", 1.0),
    )):
        off = i * 640
        w = np.asarray(inputs[wn], np.float32)       # [128 out, 512 in]
        wp[:, off:off + 512] = ws * (
            w.reshape(128, KC, 128).transpose(2, 1, 0).reshape(128, 512))
        wp[:, off + 512:off + 640] = (
            ws * np.asarray(inputs[un], np.float32).T)
        # bias order in bp: [q, r, z] (matches b_sb usage)
        g = (1, 0, 2)[i]
        bp[:, g] = (np.asarray(inputs[wbn], np.float32)
                    + np.asarray(inputs[ubn], np.float32))
    # w8 [128, k2, j, o] fp8: WRS * Wr_w[o, (2*k2+j)*128 + p]
    wr = np.asarray(inputs["Wr_w"], np.float32)      # [128, 512]
    w8 = (WRS * wr.reshape(128, 2, 2, 128).transpose(3, 1, 2, 0))
    return (np.ascontiguousarray(wp.astype(ml_dtypes.bfloat16)),
            np.ascontiguousarray(w8.astype(ml_dtypes.float8_e4m3)),
            np.ascontiguousarray(bp))


def _run(inputs, trace=False, **run_kwargs):
    xs = np.asarray(inputs["xs"], dtype=np.float32)
    zp = np.asarray(inputs["z_prev"], dtype=np.float32)
    assert xs.shape == (B, IN_DIM, L) and zp.shape == (B, WIDTH, L)
    xs_bf = np.ascontiguousarray(xs.astype(ml_dtypes.bfloat16))
    zp_bf = np.ascontiguousarray(zp.astype(ml_dtypes.bfloat16))
    wp, w8, bp = _pack_weights(inputs)

    nc = _build()
    in_maps = []
    for c in range(N_CORES):
        m = {"xs": np.ascontiguousarray(xs_bf[c * B_PER:(c + 1) * B_PER]),
             "zp": np.ascontiguousarray(zp_bf[c * B_PER:(c + 1) * B_PER]),
             "wp": wp, "w8": w8, "bp": bp}
        in_maps.append(m)

    res = run_bass_kernel_spmd(nc, in_maps, core_ids=list(range(N_CORES)),
                               trace=trace, **run_kwargs)
    out = np.concatenate(
        [np.asarray(res.results[c]["out"], dtype=np.float32)
         for c in range(N_CORES)], axis=0)
    return out, res


def kernel(**inputs):
    out, _ = _run(inputs, trace=False)
    return out


# revision 22
# speedup vs baseline: 1.0125x; 1.0125x over previous
"""GRUAggregation1d Trainium2 kernel.

Computes, for xs [B=16, 512, L=8192], z_prev [B, 128, L] (all fp32):
    q  = sigmoid(Wq@xs + Uq@z + bq)        (per position l, batch b)
    r  = sigmoid(Wr@xs + Ur@z + br)
    zt = tanh(Wz@xs + Uz@(r*z) + bz)
    out = q*z + (1-q)*zt

Sharding: data-parallel over batch. 8 cores x 2 batches each; weights
replicated.

Design (v11 ~116.7us -> this v13):
- q/zt matmuls bf16 (fp8 measured out of tolerance on those paths); the
  r gate runs fp8 DoubleRow (K=256 per pass) with x64 pre-scaled weights,
  compensated via the sigmoid's scale; Ur is pre-scaled x64 in bf16 so
  its products land on the same PSUM scale. PSUM fp32, N=512 per matmul.
- Work unit: 1024-position supertile == DMA chunk. 3-chunk input
  prefetch; xs/z on the sync HW queue, out-stores on gpsimd. The fp8
  copy of xs is derived ON-CHIP, split 3/4 on DVE (tensor-scalar cast,
  k-chunks 0-2, 1.7us) and 1/4 on ACT (copy, k-chunk 3, 1.2us): v11 did
  the whole cast on DVE (2.3us), which put DVE at 62% busy and caused a
  1.9us PE stall + HAM re-throttle when cast+rz+combine fell behind.
- Chunk 0 is latency-tuned: the weight pack is reordered [Wr|Ur|Wq|Uq|
  Wz|Uz] and DMA'd in two pieces (Wr+Ur 160KB first) so LDWEIGHTS fires
  at ~9.4us; chunk 0's xs arrives as k-quarter DMAs, k0-k2 on sync
  interleaved with z, k3 on the scalar HW queue (its only bulk use —
  the ring is ~123 GB/s, fine for a one-off 256KB). Chunk 0's r gate
  runs k-major on the bf16 Wr (x64 pre-scaled like the fp8 twin), with
  Ur accumulated mid-stream (stop on the k3 matmuls) as soon as z lands.
- Per supertile the r gate is computed FIRST so r*z (DVE, bf16) is
  ready ~1.3us before the Uz matmuls at the stream tail.
- Consecutive matmuls always target different PSUM banks; the two
  half-tile matmuls per weight are adjacent (stationary-weight reuse).
- ACT ops are 1024 wide with fused bias (PSUM fp32 in, bf16 out); the
  combine runs in bf16 on DVE (2x rate); z_prev and out are bf16 in HBM
  (host casts), halving that traffic. ACT stream per supertile: sig_r,
  sig_q, cast_k3(n+2), tanh — the cast sits in ACT's idle window, off
  the sig_r -> rz critical chain.
- The last chunk's two half-stores go to the scalar+sync HW queues in
  parallel; the gpsimd software queue (~120-150 GB/s + 2.7us drain) was
  the tail straggler.

Measured dead ends (do not retry without new evidence; noise is ~+-1us):
- fp8 (even with xs hi+lo residual splitting) on the q or tanh paths:
  2.1e-2..3.3e-2 max rel err vs the 2e-2 gate. Only the r gate tolerates
  fp8 (sigmoid'-damped, then averaged through Uz and tanh).
- DoubleRow streams 1024 column-slots per K=256 matmul on this silicon
  (~430ns); the win comes from two DR matmuls to different PSUM banks
  running concurrently (~600ns/pair = 1.44x). Permuting the rhs AP to
  [p, n, 2] fails BIR verification (pair must be dim1).
- bf16 matmuls do NOT pair-overlap (trace: strict 216ns N=512 spacing
  even across PSUM banks); the stream floor is ~5.95us/supertile.
- HOST-SHIPPED x8 (v12): +10.6us. The scalar HW ring is only ~123 GB/s
  while busy, and total input (33.6MB/core) hits the ~358 GB/s HBM/NC
  ceiling: the x8 stream starved, the PE stalled ~11us total and HAM
  re-throttled mid-kernel. x8 must be derived on-chip.
- 2048-position DMA granularity (2 supertiles/load): +9us. Emitting
  x8/xs triggers in swapped order: +5us.
- GpSimd for the bulk fp8 cast: ~70us per [128,4096] op (30x too slow).
"""

from contextlib import ExitStack

import ml_dtypes
import numpy as np

import concourse.bass as bass
import concourse.mybir as mybir
import concourse.tile as tile
from concourse import bacc
from concourse.bass_utils import run_bass_kernel_spmd

B, IN_DIM, WIDTH, L = 16, 512, 128, 8192
N_CORES = 8
B_PER = B // N_CORES          # batches per core
KC = IN_DIM // 128            # K chunks for the W matmuls
NT = 512                      # positions per matmul (one PSUM bank)
ST = 1024                     # supertile / DMA chunk positions
F32 = mybir.dt.float32
BF16 = mybir.dt.bfloat16
FP8 = mybir.dt.float8e4
WRS = 64.0                    # r-gate fp8 weight pre-scale
WP_FIRST = 640                # Wr + Ur columns, shipped first

_module_cache = {}


def _build():
    key = ("v16", ST)
    if key in _module_cache:
        return _module_cache[key]

    nc = bacc.Bacc("TRN2", target_bir_lowering=False, debug=False,
                   num_devices=N_CORES)

    xs_d = nc.dram_tensor("xs", [B_PER, IN_DIM, L], BF16, kind="ExternalInput").ap()
    zp_d = nc.dram_tensor("zp", [B_PER, WIDTH, L], BF16, kind="ExternalInput").ap()
    wp_d = nc.dram_tensor("wp", [128, 1920], BF16, kind="ExternalInput").ap()
    w8_d = nc.dram_tensor("w8", [128, 2, 2, 128], FP8, kind="ExternalInput").ap()
    bp_d = nc.dram_tensor("bp", [128, 3], F32, kind="ExternalInput").ap()
    out_d = nc.dram_tensor("out", [B_PER, WIDTH, L], BF16,
                           kind="ExternalOutput").ap()

    # [b, (k p), l] -> [b, p, k, l]: a chunk slice is a [128, KC, ST] DMA
    # with contiguous rows
    xs_r = xs_d.rearrange("b (k p) l -> b p k l", p=128)

    with tile.TileContext(nc) as tc, ExitStack() as ctx:
        wpool = ctx.enter_context(tc.tile_pool(name="weights", bufs=1))
        io = ctx.enter_context(tc.tile_pool(name="io", bufs=5))
        obuf = ctx.enter_context(tc.tile_pool(name="obuf", bufs=2))
        acts = ctx.enter_context(tc.tile_pool(name="acts", bufs=2))
        # r_ps is DOUBLE-buffered: the next supertile's DR matmuls are
        # its very first instructions, so they must never wait on the
        # previous sig_r.  q_ps/zt_ps single-buffered: their readers
        # (sig_q, tanh) finish well before the next supertile's q/zt
        # matmuls (~0.5us / ~2.6us into the stream) need the banks.
        ps_r = ctx.enter_context(tc.tile_pool(name="ps_r", bufs=2,
                                              space="PSUM"))
        ps_qz = ctx.enter_context(tc.tile_pool(name="ps_qz", bufs=1,
                                               space="PSUM"))

        w_sb = wpool.tile([128, 1920], BF16, tag="wp")
        w8_sb = wpool.tile([128, 2, 2, 128], FP8, tag="w8")
        b_sb = wpool.tile([128, 3], F32, tag="bp")
        # Wr+Ur first on sync (LDWEIGHTS at ~9.4us); w8 on gpsimd; bias
        # on scalar.  wp_rest follows chunk 0's sync-side inputs.
        nc.sync.dma_start(w_sb[:, 0:WP_FIRST], wp_d[:, 0:WP_FIRST])
        nc.gpsimd.dma_start(w8_sb[:], w8_d[:])
        nc.scalar.dma_start(b_sb[:], bp_d[:])

        # PE warmup: ~6 dummy matmuls on a zeroed scratch tile fill the
        # HAM activity window during the DMA fill, so the real stream
        # starts at 2.4 GHz instead of paying ~2.5us of 1.2 GHz matmuls
        # (HAM un-throttles only after ~3.4us of sustained PE busy).
        warm = wpool.tile([128, NT], BF16, tag="warm")
        nc.vector.memzero(warm[:])
        warm_ps = ps_qz.tile([128, ST], F32, tag="q_ps")
        for i in range(11):
            h = i % 2
            nc.tensor.matmul(warm_ps[:, h * NT:(h + 1) * NT],
                             warm[:, 0:128], warm[:], start=True, stop=True)

        # wp layout: [Wr 512 | Ur 128 | Wq 512 | Uq 128 | Wz 512 | Uz 128]
        def wr_slice(k):
            return w_sb[:, k * 128:(k + 1) * 128]

        def wq_slice(k):
            return w_sb[:, 640 + k * 128: 640 + (k + 1) * 128]

        def wz_slice(k):
            return w_sb[:, 1280 + k * 128: 1280 + (k + 1) * 128]

        def uslice(u):  # 0=r, 1=q, 2=z
            off = (512, 1152, 1792)[u]
            return w_sb[:, off: off + 128]

        n_chunks = B_PER * (L // ST)
        tiles = {}

        def load_chunk(m):
            """Input DMAs for chunk m. The fp8 tile is allocated here but
            filled later by DVE (k0-2) + ACT (k3) casts, ~2 chunks ahead
            of use."""
            mb, ml = divmod(m, L // ST)
            ml *= ST
            xs_t = io.tile([128, KC, ST], BF16, tag="xs_t")
            z_t = io.tile([128, ST], BF16, tag="z_t")
            if m == 0:
                # chunk 0 is DMA-sem-paced (~2us completion latency per
                # piece, early queue ~130 GB/s): k0, k1 + z on sync with
                # wp_rest slotted between, k2+k3 on the scalar HW queue.
                nc.sync.dma_start(xs_t[:, 0], xs_r[mb][:, 0, ml:ml + ST])
                nc.sync.dma_start(w_sb[:, WP_FIRST:1920],
                                  wp_d[:, WP_FIRST:1920])
                nc.sync.dma_start(xs_t[:, 1], xs_r[mb][:, 1, ml:ml + ST])
                nc.sync.dma_start(z_t[:], zp_d[mb][:, ml:ml + ST])
                for k in (2, 3):
                    nc.scalar.dma_start(xs_t[:, k],
                                        xs_r[mb][:, k, ml:ml + ST])
            else:
                nc.sync.dma_start(xs_t[:], xs_r[mb][:, :, ml:ml + ST])
                nc.sync.dma_start(z_t[:], zp_d[mb][:, ml:ml + ST])
            x8_t = None
            if m > 1:  # chunks 0-1's r gate runs on the bf16 weights
                x8_t = io.tile([128, KC, ST], FP8, tag="x8_t")
            tiles[m] = (xs_t, x8_t, z_t)

        LOOKAHEAD = 2
        for m in range(min(LOOKAHEAD + 1, n_chunks)):
            load_chunk(m)

        for n in range(n_chunks):
            if n + LOOKAHEAD + 1 < n_chunks:
                load_chunk(n + LOOKAHEAD + 1)
            b_i, l0 = divmod(n, L // ST)
            l0 *= ST
            xs_t, x8_t, z_t = tiles.pop(n)
            ob = obuf.tile([128, ST], BF16, tag="ob")

            q_ps = ps_qz.tile([128, ST], F32, tag="q_ps")
            r_ps = ps_r.tile([128, ST], F32, tag="r_ps")
            zt_ps = ps_qz.tile([128, ST], F32, tag="zt_ps")

            def r_w_bf16(k, start):
                for h in range(2):
                    nc.tensor.matmul(r_ps[:, h * NT:(h + 1) * NT],
                                     wr_slice(k),
                                     xs_t[:, k, h * NT:(h + 1) * NT],
                                     start=start, stop=False)

            def q_w(k):
                for h in range(2):
                    nc.tensor.matmul(q_ps[:, h * NT:(h + 1) * NT],
                                     wq_slice(k),
                                     xs_t[:, k, h * NT:(h + 1) * NT],
                                     start=(k == 0), stop=False)

            def u_mm(ps, u, h, rhs, stop=True):
                nc.tensor.matmul(ps[:, h * NT:(h + 1) * NT], uslice(u),
                                 rhs[:, h * NT:(h + 1) * NT],
                                 start=False, stop=stop)

            if n == 0:
                # ordered by measured DMA-completion times: sync carries
                # [wp_f, k0, k1, wp_rest, z] (sems ~12.5/13.6/15.3/16.4
                # us), scalar [bp, k2, k3] (~14.6/16.6us)
                r_w_bf16(0, True)
                r_w_bf16(1, False)
                r_w_bf16(2, False)
                q_w(0)
                q_w(1)
                q_w(2)
                r_w_bf16(3, False)
                q_w(3)
                for h in range(2):
                    u_mm(r_ps, 0, h, z_t)
                for h in range(2):
                    u_mm(q_ps, 1, h, z_t)
            elif n == 1:
                # bf16 r as well: skipping the x8 cast for chunk 1 keeps
                # DVE's chunk-0 window free and avoids a cast-gated stall
                # at the 0->1 boundary.
                for k in range(KC):
                    r_w_bf16(k, k == 0)
                for h in range(2):
                    u_mm(r_ps, 0, h, z_t)
                for k in range(KC):
                    q_w(k)
                for h in range(2):
                    u_mm(q_ps, 1, h, z_t)
            else:
                # The PE dual-issues a DoubleRow matmul alongside an
                # in-flight bf16 matmul when they target different PSUM
                # banks (~215ns marginal cost vs 600ns/pair standalone).
                # Weave the 4 r DR matmuls into the q stream, with each
                # Ur emitted right after its bank's last DR so sig_r can
                # release r_ps early for the next supertile.
                dr = [(0, 0), (0, 1), (1, 0), (1, 1)]
                for k in range(KC):
                    k2, dh = dr[k]
                    nc.tensor.matmul(
                        r_ps[:, dh * NT:(dh + 1) * NT], w8_sb[:, k2],
                        x8_t[:, 2 * k2:2 * k2 + 2, dh * NT:(dh + 1) * NT],
                        start=(k2 == 0), stop=False,
                        perf_mode=mybir.MatmulPerfMode.DoubleRow)
                    if k < 3:
                        q_w(k)
                    if k == 2:
                        u_mm(r_ps, 0, 0, z_t)   # bank A stop after DR10
                u_mm(r_ps, 0, 1, z_t)           # bank B stop after DR11
                q_w(3)
                for h in range(2):
                    u_mm(q_ps, 1, h, z_t)
            r_s = acts.tile([128, ST], BF16, tag="r_s")
            nc.scalar.activation(r_s[:], r_ps[:],
                                 mybir.ActivationFunctionType.Sigmoid,
                                 bias=b_sb[:, 1:2], scale=1.0 / WRS)
            rz = acts.tile([128, ST], BF16, tag="rz")
            nc.vector.tensor_mul(rz[:], r_s[:], z_t[:])
            # bf16->fp8 cast of chunk n+2's xs, split DVE (k0-2) + ACT
            # (k3), each in its idle window, always >=2 chunks ahead of
            # the fp8 r matmuls that consume it.
            cm = n + 2
            cast = (cm < n_chunks
                    and tiles.get(cm, (None, None))[1] is not None)
            if cast:
                nc.vector.tensor_scalar_add(tiles[cm][1][:, 0:3],
                                            tiles[cm][0][:, 0:3], 0.0)
            q_s = acts.tile([128, ST], BF16, tag="q_s")
            nc.scalar.activation(q_s[:], q_ps[:],
                                 mybir.ActivationFunctionType.Sigmoid,
                                 bias=b_sb[:, 0:1])
            if cast:
                nc.scalar.copy(tiles[cm][1][:, 3], tiles[cm][0][:, 3])

            # ---- zt gate: W part, then Uz@(r*z) at the stream tail
            for k in range(KC):
                w = wz_slice(k)
                for h in range(2):
                    nc.tensor.matmul(
                        zt_ps[:, h * NT:(h + 1) * NT], w,
                        xs_t[:, k, h * NT:(h + 1) * NT],
                        start=(k == 0), stop=False)
            uz = uslice(2)
            for h in range(2):
                nc.tensor.matmul(zt_ps[:, h * NT:(h + 1) * NT], uz,
                                 rz[:, h * NT:(h + 1) * NT],
                                 start=False, stop=True)

            # ---- epilogue: out = zt + q*(z - zt), bf16 on DVE. The last
            # chunk runs it in 512-halves so the serial tail chain
            # (tanh -> sub -> mul -> add -> store) pipelines, with the
            # half-stores split over the two fast HW queues.
            if n < n_chunks - 1:
                pieces = ((0, ST),)
            else:
                pieces = ((0, NT), (NT, NT))
            for pi, (h0, hw_) in enumerate(pieces):
                zt_s = acts.tile([128, hw_], BF16, tag=f"zt_s{h0}")
                nc.scalar.activation(zt_s[:], zt_ps[:, h0:h0 + hw_],
                                     mybir.ActivationFunctionType.Tanh,
                                     bias=b_sb[:, 2:3])
                diff = acts.tile([128, hw_], BF16, tag=f"diff{h0}")
                nc.vector.tensor_sub(diff[:], z_t[:, h0:h0 + hw_], zt_s[:])
                prod = acts.tile([128, hw_], BF16, tag=f"prod{h0}")
                nc.vector.tensor_mul(prod[:], q_s[:, h0:h0 + hw_], diff[:])
                nc.vector.tensor_add(ob[:, h0:h0 + hw_], zt_s[:], prod[:])
                if n < n_chunks - 1:
                    eng = nc.gpsimd
                else:
                    eng = nc.scalar if pi % 2 == 0 else nc.sync
                eng.dma_start(out_d[b_i][:, l0 + h0:l0 + h0 + hw_],
                              ob[:, h0:h0 + hw_])

    nc.compile()
    _module_cache[key] = nc
    return nc


def _pack_weights(inputs):
    # wp [128, 1920] bf16, layout [Wr|Ur|Wq|Uq|Wz|Uz]; per partition p:
    #   W at [p, off+k*128+o] = Wg_w[o, k*128+p]; U at [p, off+o] =
    #   Ug_w[o, p].  Wr/Ur pre-scaled by WRS (fp8-path compensation).
    wp = np.empty((128, 1920), np.float32)
    bp = np.empty((128, 3), np.float32)
    for i, (wn, un, wbn, ubn, ws) in enumerate((
        ("Wr_w", "Ur_w", "Wr_b", "Ur_b", WRS),
        ("Wq_w", "Uq_w", "Wq_b", "Uq_b", 1.0),
        ("Wz_w", "Uz_w", "Wz_b", "Uz_b", 1.0),
    )):
        off = i * 640
        w = np.asarray(inputs[wn], np.float32)       # [128 out, 512 in]
        wp[:, off:off + 512] = ws * (
            w.reshape(128, KC, 128).transpose(2, 1, 0).reshape(128, 512))
        wp[:, off + 512:off + 640] = (
            ws * np.asarray(inputs[un], np.float32).T)
        # bias order in bp: [q, r, z] (matches b_sb usage)
        g = (1, 0, 2)[i]
        bp[:, g] = (np.asarray(inputs[wbn], np.float32)
                    + np.asarray(inputs[ubn], np.float32))
    # w8 [128, k2, j, o] fp8: WRS * Wr_w[o, (2*k2+j)*128 + p]
    wr = np.asarray(inputs["Wr_w"], np.float32)      # [128, 512]
    w8 = (WRS * wr.reshape(128, 2, 2, 128).transpose(3, 1, 2, 0))
    return (np.ascontiguousarray(wp.astype(ml_dtypes.bfloat16)),
            np.ascontiguousarray(w8.astype(ml_dtypes.float8_e4m3)),
            np.ascontiguousarray(bp))


def _run(inputs, trace=False, **run_kwargs):
    xs = np.asarray(inputs["xs"], dtype=np.float32)
    zp = np.asarray(inputs["z_prev"], dtype=np.float32)
    assert xs.shape == (B, IN_DIM, L) and zp.shape == (B, WIDTH, L)
    xs_bf = np.ascontiguousarray(xs.astype(ml_dtypes.bfloat16))
    zp_bf = np.ascontiguousarray(zp.astype(ml_dtypes.bfloat16))
    wp, w8, bp = _pack_weights(inputs)

    nc = _build()
    in_maps = []
    for c in range(N_CORES):
        m = {"xs": np.ascontiguousarray(xs_bf[c * B_PER:(c + 1) * B_PER]),
             "zp": np.ascontiguousarray(zp_bf[c * B_PER:(c + 1) * B_PER]),
             "wp": wp, "w8": w8, "bp": bp}
        in_maps.append(m)

    res = run_bass_kernel_spmd(nc, in_maps, core_ids=list(range(N_CORES)),
                               trace=trace, **run_kwargs)
    out = np.concatenate(
        [np.asarray(res.results[c]["out"], dtype=np.float32)
         for c in range(N_CORES)], axis=0)
    return out, res


def kernel(**inputs):
    out, _ = _run(inputs, trace=False)
    return out


# revision 27
# speedup vs baseline: 1.0402x; 1.0274x over previous
"""GRUAggregation1d Trainium2 kernel.

Computes, for xs [B=16, 512, L=8192], z_prev [B, 128, L] (all fp32):
    q  = sigmoid(Wq@xs + Uq@z + bq)        (per position l, batch b)
    r  = sigmoid(Wr@xs + Ur@z + br)
    zt = tanh(Wz@xs + Uz@(r*z) + bz)
    out = q*z + (1-q)*zt

Sharding: data-parallel over batch. 8 cores x 2 batches each; weights
replicated.

Design (v11 ~116.7us -> this v13):
- q/zt matmuls bf16 (fp8 measured out of tolerance on those paths); the
  r gate runs fp8 DoubleRow (K=256 per pass) with x64 pre-scaled weights,
  compensated via the sigmoid's scale; Ur is pre-scaled x64 in bf16 so
  its products land on the same PSUM scale. PSUM fp32, N=512 per matmul.
- Work unit: 1024-position supertile == DMA chunk. 3-chunk input
  prefetch; xs/z on the sync HW queue, out-stores on gpsimd. The fp8
  copy of xs is derived ON-CHIP, split 3/4 on DVE (tensor-scalar cast,
  k-chunks 0-2, 1.7us) and 1/4 on ACT (copy, k-chunk 3, 1.2us): v11 did
  the whole cast on DVE (2.3us), which put DVE at 62% busy and caused a
  1.9us PE stall + HAM re-throttle when cast+rz+combine fell behind.
- Chunk 0 is latency-tuned: the weight pack is reordered [Wr|Ur|Wq|Uq|
  Wz|Uz] and DMA'd in two pieces (Wr+Ur 160KB first) so LDWEIGHTS fires
  at ~9.4us; chunk 0's xs arrives as k-quarter DMAs, k0-k2 on sync
  interleaved with z, k3 on the scalar HW queue (its only bulk use —
  the ring is ~123 GB/s, fine for a one-off 256KB). Chunk 0's r gate
  runs k-major on the bf16 Wr (x64 pre-scaled like the fp8 twin), with
  Ur accumulated mid-stream (stop on the k3 matmuls) as soon as z lands.
- Per supertile the r gate is computed FIRST so r*z (DVE, bf16) is
  ready ~1.3us before the Uz matmuls at the stream tail.
- Consecutive matmuls always target different PSUM banks; the two
  half-tile matmuls per weight are adjacent (stationary-weight reuse).
- ACT ops are 1024 wide with fused bias (PSUM fp32 in, bf16 out); the
  combine runs in bf16 on DVE (2x rate); z_prev and out are bf16 in HBM
  (host casts), halving that traffic. ACT stream per supertile: sig_r,
  sig_q, cast_k3(n+2), tanh — the cast sits in ACT's idle window, off
  the sig_r -> rz critical chain.
- The last chunk's two half-stores go to the scalar+sync HW queues in
  parallel; the gpsimd software queue (~120-150 GB/s + 2.7us drain) was
  the tail straggler.

Measured dead ends (do not retry without new evidence; noise is ~+-1us):
- fp8 (even with xs hi+lo residual splitting) on the q or tanh paths:
  2.1e-2..3.3e-2 max rel err vs the 2e-2 gate. Only the r gate tolerates
  fp8 (sigmoid'-damped, then averaged through Uz and tanh).
- DoubleRow streams 1024 column-slots per K=256 matmul on this silicon
  (~430ns); the win comes from two DR matmuls to different PSUM banks
  running concurrently (~600ns/pair = 1.44x). Permuting the rhs AP to
  [p, n, 2] fails BIR verification (pair must be dim1).
- bf16 matmuls do NOT pair-overlap (trace: strict 216ns N=512 spacing
  even across PSUM banks); the stream floor is ~5.95us/supertile.
- HOST-SHIPPED x8 (v12): +10.6us. The scalar HW ring is only ~123 GB/s
  while busy, and total input (33.6MB/core) hits the ~358 GB/s HBM/NC
  ceiling: the x8 stream starved, the PE stalled ~11us total and HAM
  re-throttled mid-kernel. x8 must be derived on-chip.
- 2048-position DMA granularity (2 supertiles/load): +9us. Emitting
  x8/xs triggers in swapped order: +5us.
- GpSimd for the bulk fp8 cast: ~70us per [128,4096] op (30x too slow).
"""

from contextlib import ExitStack

import ml_dtypes
import numpy as np

import concourse.bass as bass
import concourse.mybir as mybir
import concourse.tile as tile
from concourse import bacc
from concourse.bass_utils import run_bass_kernel_spmd

B, IN_DIM, WIDTH, L = 16, 512, 128, 8192
N_CORES = 8
B_PER = B // N_CORES          # batches per core
KC = IN_DIM // 128            # K chunks for the W matmuls
NT = 512                      # positions per matmul (one PSUM bank)
ST = 1024                     # supertile / DMA chunk positions
F32 = mybir.dt.float32
BF16 = mybir.dt.bfloat16
FP8 = mybir.dt.float8e4
WRS = 64.0                    # r-gate fp8 weight pre-scale
WP_FIRST = 640                # Wr + Ur columns, shipped first

_module_cache = {}


def _build():
    key = ("v17", ST)
    if key in _module_cache:
        return _module_cache[key]

    nc = bacc.Bacc("TRN2", target_bir_lowering=False, debug=False,
                   num_devices=N_CORES)

    xs_d = nc.dram_tensor("xs", [B_PER, IN_DIM, L], BF16, kind="ExternalInput").ap()
    zp_d = nc.dram_tensor("zp", [B_PER, WIDTH, L], BF16, kind="ExternalInput").ap()
    wp_d = nc.dram_tensor("wp", [128, 1920], BF16, kind="ExternalInput").ap()
    w8_d = nc.dram_tensor("w8", [128, 2, 2, 128], FP8, kind="ExternalInput").ap()
    bp_d = nc.dram_tensor("bp", [128, 3], F32, kind="ExternalInput").ap()
    out_d = nc.dram_tensor("out", [B_PER, WIDTH, L], BF16,
                           kind="ExternalOutput").ap()

    # [b, (k p), l] -> [b, p, k, l]: a chunk slice is a [128, KC, ST] DMA
    # with contiguous rows
    xs_r = xs_d.rearrange("b (k p) l -> b p k l", p=128)

    with tile.TileContext(nc) as tc, ExitStack() as ctx:
        wpool = ctx.enter_context(tc.tile_pool(name="weights", bufs=1))
        io = ctx.enter_context(tc.tile_pool(name="io", bufs=5))
        obuf = ctx.enter_context(tc.tile_pool(name="obuf", bufs=2))
        acts = ctx.enter_context(tc.tile_pool(name="acts", bufs=2))
        # r_ps is DOUBLE-buffered: the next supertile's DR matmuls are
        # its very first instructions, so they must never wait on the
        # previous sig_r.  q_ps/zt_ps single-buffered: their readers
        # (sig_q, tanh) finish well before the next supertile's q/zt
        # matmuls (~0.5us / ~2.6us into the stream) need the banks.
        ps_r = ctx.enter_context(tc.tile_pool(name="ps_r", bufs=2,
                                              space="PSUM"))
        ps_qz = ctx.enter_context(tc.tile_pool(name="ps_qz", bufs=1,
                                               space="PSUM"))

        w_sb = wpool.tile([128, 1920], BF16, tag="wp")
        w8_sb = wpool.tile([128, 2, 2, 128], FP8, tag="w8")
        b_sb = wpool.tile([128, 3], F32, tag="bp")
        # Wr+Ur first on sync (LDWEIGHTS at ~9.4us); w8 on gpsimd; bias
        # on scalar.  wp_rest follows chunk 0's sync-side inputs.
        nc.sync.dma_start(w_sb[:, 0:WP_FIRST], wp_d[:, 0:WP_FIRST])
        nc.gpsimd.dma_start(w8_sb[:], w8_d[:])
        nc.scalar.dma_start(b_sb[:], bp_d[:])

        # (PE warmup matmuls were tried and dropped: chunk 0's ~1-2us
        # DMA-completion gaps re-throttle HAM regardless, and chunk 0's
        # end time is pinned by the wp_rest DMA sem, not by PE clock.)

        # wp layout: [Wr 512 | Ur 128 | Wq 512 | Uq 128 | Wz 512 | Uz 128]
        def wr_slice(k):
            return w_sb[:, k * 128:(k + 1) * 128]

        def wq_slice(k):
            return w_sb[:, 640 + k * 128: 640 + (k + 1) * 128]

        def wz_slice(k):
            return w_sb[:, 1280 + k * 128: 1280 + (k + 1) * 128]

        def uslice(u):  # 0=r, 1=q, 2=z
            off = (512, 1152, 1792)[u]
            return w_sb[:, off: off + 128]

        n_chunks = B_PER * (L // ST)
        tiles = {}

        def load_chunk(m):
            """Input DMAs for chunk m. The fp8 tile is allocated here but
            filled later by DVE (k0-2) + ACT (k3) casts, ~2 chunks ahead
            of use."""
            mb, ml = divmod(m, L // ST)
            ml *= ST
            xs_t = io.tile([128, KC, ST], BF16, tag="xs_t")
            z_t = io.tile([128, ST], BF16, tag="z_t")
            if m == 0:
                # chunk 0 is DMA-sem-paced (~1-2us completion latency
                # per piece, early sync queue ~130-150 GB/s): k-quarters
                # k0-k2 + z on sync, k3 on the scalar HW queue, wp_rest
                # last (its sem pins the q-gate start ~17.9us).
                for k in range(2):
                    nc.sync.dma_start(xs_t[:, k], xs_r[mb][:, k, ml:ml + ST])
                nc.sync.dma_start(z_t[:], zp_d[mb][:, ml:ml + ST])
                nc.sync.dma_start(xs_t[:, 2], xs_r[mb][:, 2, ml:ml + ST])
                nc.scalar.dma_start(xs_t[:, 3], xs_r[mb][:, 3, ml:ml + ST])
                nc.sync.dma_start(w_sb[:, WP_FIRST:1920],
                                  wp_d[:, WP_FIRST:1920])
            else:
                nc.sync.dma_start(xs_t[:], xs_r[mb][:, :, ml:ml + ST])
                nc.sync.dma_start(z_t[:], zp_d[mb][:, ml:ml + ST])
            x8_t = None
            if m > 0:  # chunk 0's r gate runs on the bf16 weights
                x8_t = io.tile([128, KC, ST], FP8, tag="x8_t")
            tiles[m] = (xs_t, x8_t, z_t)

        LOOKAHEAD = 2
        for m in range(min(LOOKAHEAD + 1, n_chunks)):
            load_chunk(m)

        for n in range(n_chunks):
            if n + LOOKAHEAD + 1 < n_chunks:
                load_chunk(n + LOOKAHEAD + 1)
            b_i, l0 = divmod(n, L // ST)
            l0 *= ST
            xs_t, x8_t, z_t = tiles.pop(n)
            ob = obuf.tile([128, ST], BF16, tag="ob")

            q_ps = ps_qz.tile([128, ST], F32, tag="q_ps")
            r_ps = ps_r.tile([128, ST], F32, tag="r_ps")
            zt_ps = ps_qz.tile([128, ST], F32, tag="zt_ps")

            def r_w_bf16(k, start):
                for h in range(2):
                    nc.tensor.matmul(r_ps[:, h * NT:(h + 1) * NT],
                                     wr_slice(k),
                                     xs_t[:, k, h * NT:(h + 1) * NT],
                                     start=start, stop=False)

            def q_w(k):
                for h in range(2):
                    nc.tensor.matmul(q_ps[:, h * NT:(h + 1) * NT],
                                     wq_slice(k),
                                     xs_t[:, k, h * NT:(h + 1) * NT],
                                     start=(k == 0), stop=False)

            def u_mm(ps, u, h, rhs, stop=True):
                nc.tensor.matmul(ps[:, h * NT:(h + 1) * NT], uslice(u),
                                 rhs[:, h * NT:(h + 1) * NT],
                                 start=False, stop=stop)

            if n == 0:
                # bf16 Wr, ordered by measured DMA-completion times
                # (sync k0 ~12.3, k1 ~14.3, z ~15.1, k2 ~16.4; scalar
                # k3 ~17.0; wp_rest ~17.9 gates the q gate).
                r_w_bf16(0, True)
                r_w_bf16(1, False)
                for h in range(2):
                    u_mm(r_ps, 0, h, z_t, stop=False)
                r_w_bf16(2, False)
                for h in range(2):
                    nc.tensor.matmul(r_ps[:, h * NT:(h + 1) * NT],
                                     wr_slice(3),
                                     xs_t[:, 3, h * NT:(h + 1) * NT],
                                     start=False, stop=True)
                for k in range(KC):
                    q_w(k)
                for h in range(2):
                    u_mm(q_ps, 1, h, z_t)
            else:
                # The PE dual-issues a DoubleRow matmul alongside an
                # in-flight bf16 matmul when they target different PSUM
                # banks (~215ns marginal cost vs 600ns/pair standalone).
                # Weave the 4 r DR matmuls into the q stream, with each
                # Ur emitted right after its bank's last DR so sig_r can
                # release r_ps early for the next supertile.
                dr = [(0, 0), (0, 1), (1, 0), (1, 1)]
                for k in range(KC):
                    k2, dh = dr[k]
                    nc.tensor.matmul(
                        r_ps[:, dh * NT:(dh + 1) * NT], w8_sb[:, k2],
                        x8_t[:, 2 * k2:2 * k2 + 2, dh * NT:(dh + 1) * NT],
                        start=(k2 == 0), stop=False,
                        perf_mode=mybir.MatmulPerfMode.DoubleRow)
                    if k < 3:
                        q_w(k)
                    if k == 2:
                        u_mm(r_ps, 0, 0, z_t)   # bank A stop after DR10
                u_mm(r_ps, 0, 1, z_t)           # bank B stop after DR11
                q_w(3)
                for h in range(2):
                    u_mm(q_ps, 1, h, z_t)
            r_s = acts.tile([128, ST], BF16, tag="r_s")
            nc.scalar.activation(r_s[:], r_ps[:],
                                 mybir.ActivationFunctionType.Sigmoid,
                                 bias=b_sb[:, 1:2], scale=1.0 / WRS)
            rz = acts.tile([128, ST], BF16, tag="rz")
            nc.vector.tensor_mul(rz[:], r_s[:], z_t[:])
            # bf16->fp8 cast of chunk n+2's xs, split DVE (k0-2) + ACT
            # (k3), each in its idle window, always >=2 chunks ahead of
            # the fp8 r matmuls that consume it.  n=0 additionally casts
            # chunk 1 entirely on DVE (its DRs run ~2 chunks early).
            if n == 0 and tiles.get(1, (None, None))[1] is not None:
                nc.vector.tensor_scalar_add(tiles[1][1][:],
                                            tiles[1][0][:], 0.0)
            cm = n + 2
            cast = (cm < n_chunks
                    and tiles.get(cm, (None, None))[1] is not None)
            if cast:
                nc.vector.tensor_scalar_add(tiles[cm][1][:, 0:3],
                                            tiles[cm][0][:, 0:3], 0.0)
            q_s = acts.tile([128, ST], BF16, tag="q_s")
            nc.scalar.activation(q_s[:], q_ps[:],
                                 mybir.ActivationFunctionType.Sigmoid,
                                 bias=b_sb[:, 0:1])
            if cast:
                nc.scalar.copy(tiles[cm][1][:, 3], tiles[cm][0][:, 3])

            # ---- zt gate: W part, then Uz@(r*z) at the stream tail
            for k in range(KC):
                w = wz_slice(k)
                for h in range(2):
                    nc.tensor.matmul(
                        zt_ps[:, h * NT:(h + 1) * NT], w,
                        xs_t[:, k, h * NT:(h + 1) * NT],
                        start=(k == 0), stop=False)
            uz = uslice(2)
            for h in range(2):
                nc.tensor.matmul(zt_ps[:, h * NT:(h + 1) * NT], uz,
                                 rz[:, h * NT:(h + 1) * NT],
                                 start=False, stop=True)

            # ---- epilogue: out = zt + q*(z - zt), bf16 on DVE. The last
            # chunk runs it in 512-halves so the serial tail chain
            # (tanh -> sub -> mul -> add -> store) pipelines, with the
            # half-stores split over the two fast HW queues.
            if n < n_chunks - 1:
                pieces = ((0, ST),)
            else:
                pieces = ((0, NT), (NT, NT))
            for pi, (h0, hw_) in enumerate(pieces):
                zt_s = acts.tile([128, hw_], BF16, tag=f"zt_s{h0}")
                nc.scalar.activation(zt_s[:], zt_ps[:, h0:h0 + hw_],
                                     mybir.ActivationFunctionType.Tanh,
                                     bias=b_sb[:, 2:3])
                diff = acts.tile([128, hw_], BF16, tag=f"diff{h0}")
                nc.vector.tensor_sub(diff[:], z_t[:, h0:h0 + hw_], zt_s[:])
                prod = acts.tile([128, hw_], BF16, tag=f"prod{h0}")
                nc.vector.tensor_mul(prod[:], q_s[:, h0:h0 + hw_], diff[:])
                nc.vector.tensor_add(ob[:, h0:h0 + hw_], zt_s[:], prod[:])
                if n < n_chunks - 1:
                    eng = nc.gpsimd
                else:
                    eng = nc.scalar if pi % 2 == 0 else nc.sync
                eng.dma_start(out_d[b_i][:, l0 + h0:l0 + h0 + hw_],
                              ob[:, h0:h0 + hw_])

    nc.compile()
    _module_cache[key] = nc
    return nc


def _pack_weights(inputs):
    # wp [128, 1920] bf16, layout [Wr|Ur|Wq|Uq|Wz|Uz]; per partition p:
    #   W at [p, off+k*128+o] = Wg_w[o, k*128+p]; U at [p, off+o] =
    #   Ug_w[o, p].  Wr/Ur pre-scaled by WRS (fp8-path compensation).
    wp = np.empty((128, 1920), np.float32)
    bp = np.empty((128, 3), np.float32)
    for i, (wn, un, wbn, ubn, ws) in enumerate((
        ("Wr_w", "Ur_w", "Wr_b", "Ur_b", WRS),
        ("Wq_w", "Uq_w", "Wq_b", "Uq_b", 1.0),
        ("Wz_w", "Uz_w", "Wz_b", "Uz_b", 1.0),
    )):
        off = i * 640
        w = np.asarray(inputs[wn], np.float32)       # [128 out, 512 in]
        wp[:, off:off + 512] = ws * (
            w.reshape(128, KC, 128).transpose(2, 1, 0).reshape(128, 512))
        wp[:, off + 512:off + 640] = (
            ws * np.asarray(inputs[un], np.float32).T)
        # bias order in bp: [q, r, z] (matches b_sb usage)
        g = (1, 0, 2)[i]
        bp[:, g] = (np.asarray(inputs[wbn], np.float32)
                    + np.asarray(inputs[ubn], np.float32))
    # w8 [128, k2, j, o] fp8: WRS * Wr_w[o, (2*k2+j)*128 + p]
    wr = np.asarray(inputs["Wr_w"], np.float32)      # [128, 512]
    w8 = (WRS * wr.reshape(128, 2, 2, 128).transpose(3, 1, 2, 0))
    return (np.ascontiguousarray(wp.astype(ml_dtypes.bfloat16)),
            np.ascontiguousarray(w8.astype(ml_dtypes.float8_e4m3)),
            np.ascontiguousarray(bp))


def _run(inputs, trace=False, **run_kwargs):
    xs = np.asarray(inputs["xs"], dtype=np.float32)
    zp = np.asarray(inputs["z_prev"], dtype=np.float32)
    assert xs.shape == (B, IN_DIM, L) and zp.shape == (B, WIDTH, L)
    xs_bf = np.ascontiguousarray(xs.astype(ml_dtypes.bfloat16))
    zp_bf = np.ascontiguousarray(zp.astype(ml_dtypes.bfloat16))
    wp, w8, bp = _pack_weights(inputs)

    nc = _build()
    in_maps = []
    for c in range(N_CORES):
        m = {"xs": np.ascontiguousarray(xs_bf[c * B_PER:(c + 1) * B_PER]),
             "zp": np.ascontiguousarray(zp_bf[c * B_PER:(c + 1) * B_PER]),
             "wp": wp, "w8": w8, "bp": bp}
        in_maps.append(m)

    res = run_bass_kernel_spmd(nc, in_maps, core_ids=list(range(N_CORES)),
                               trace=trace, **run_kwargs)
    out = np.concatenate(
        [np.asarray(res.results[c]["out"], dtype=np.float32)
         for c in range(N_CORES)], axis=0)
    return out, res


def kernel(**inputs):
    out, _ = _run(inputs, trace=False)
    return out
